# revision 1
# baseline (speedup 1.0000x reference)
"""Trainium2 Bass kernel for nn_MetaBaseline (global-cosine + DN4 few-shot scoring).

Math (per episode b):
  global: logits[q,k] = <qmean_hat, bmean_hat>          (means over the 5x5 spatial grid)
  DN4:    sim[q,p,k,l] = <q_patch[q,p], s_col_hat[k,l]>  -> sum of top-neighbor_k over l,
          summed over p, / neighbor_k
  out = r0 * logits + r1 * dn4

Device strategy (data-parallel, 8 episodes per NeuronCore):
  - host pre-normalizes the support side and appends the 5 class-mean columns:
    s_ext [640, 130] per episode; query laid out as q_mat [640, 1920] (qp-major,
    zero-padded from 1875); both bf16.
  - host normalizes the query patches too (q_hat), so the device does no scaling;
    the class-mean projections (cols 125:130) then carry a spurious 1/||q_patch||
    factor that the host-built A*||q_patch|| aggregation matrix undoes.
  - PE: sim_ext[qp, 0:130] = q_hat^T @ s_ext as 15 qp-tiles x 5 k-tiles of
    [128,128]x[128,130] bf16 matmuls; two qp-tiles share one fp32 PSUM bank
    [128,260] so each PSUM->SBUF copy (split between ACT and DVE) covers two.
  - DVE Max8 gives the top-8 of each 25-value support-patch group in one op;
    one strided reduce_sum of the first neighbor_k per episode gives the
    per-(patch,class) DN4 terms.
  - tiny matmuls against the aggregation matrices contract the 25 patches of
    each query across partitions (DN4 against the 0/1 matrix, globals against
    A*||q_patch||).
  - host applies 1/(25*||q_mean||), neighbor_k, and the r-weighted combine.
"""
import numpy as np
import ml_dtypes

N_CORES = 8
B, WAY, SHOT, D, H, W = 64, 5, 1, 640, 5, 5
NQ = 75
HW = H * W                 # 25
QP = NQ * HW               # 1875 query patches per episode
NT = 15                    # qp tiles of 128
QP_PAD = NT * 128          # 1920
ND = D // 128              # 5 contraction tiles
EPC = B // N_CORES         # 8 episodes per core
SCOLS = WAY * HW + WAY     # 130
GEPS = 1e-12               # eps of the global-cosine branch (torch F.normalize)

_CACHE = {}
_LAST_IN_MAPS = None


def _build(k: int):
    """Build + compile the SPMD NEFF for top-k = k (k <= 8)."""
    import concourse.bacc as bacc
    import concourse.mybir as mybir
    import concourse.tile as tile

    bf16 = mybir.dt.bfloat16
    f32 = mybir.dt.float32
    COPY = mybir.ActivationFunctionType.Copy

    nc = bacc.Bacc("TRN2", target_bir_lowering=False, debug=False)
    qm = nc.dram_tensor("qm", [EPC, ND, 128, QP_PAD], bf16, kind="ExternalInput")
    se = nc.dram_tensor("se", [ND, 128, EPC * SCOLS], bf16, kind="ExternalInput")
    amat = nc.dram_tensor("amat", [128, NT * NQ], bf16, kind="ExternalInput")
    am2 = nc.dram_tensor("am2", [128, EPC * NT * NQ], bf16, kind="ExternalInput")
    out = nc.dram_tensor("out", [EPC, WAY, 2 * NQ], f32, kind="ExternalOutput")

    with tile.TileContext(nc) as tc:
        with (
            tc.tile_pool(name="const", bufs=1) as cpool,
            tc.tile_pool(name="q", bufs=4 * ND) as qpool,
            tc.tile_pool(name="simps", bufs=4, space="PSUM") as simpool,
            tc.tile_pool(name="acc", bufs=2, space="PSUM") as accpool,
            tc.tile_pool(name="simsb", bufs=14) as sbpool,
            tc.tile_pool(name="out8", bufs=3) as o8pool,
            tc.tile_pool(name="draw", bufs=3) as drpool,
            tc.tile_pool(name="osb", bufs=2) as opool,
        ):
            sts = []
            for d in range(ND):
                st = cpool.tile([128, EPC * SCOLS], bf16, tag=f"se{d}")
                (nc.sync if d % 2 == 0 else nc.scalar).dma_start(st[:], se[d])
                sts.append(st)
            amat_t = cpool.tile([128, NT * NQ], bf16)
            am2_t = cpool.tile([128, EPC * NT * NQ], bf16)

            pending = []  # deferred tail: (e, draw, dn4_ps, glob_ps)

            def emit_tail():
                if not pending:
                    return
                e, draw, dn4_ps, glob_ps, simsbs = pending.pop()
                if e != EPC - 1:
                    for t in range(NT):
                        simsb, off = simsbs[t]
                        nc.tensor.matmul(
                            glob_ps[:], simsb[:, off + WAY * HW:off + SCOLS],
                            am2_t[:, (e * NT + t) * NQ:(e * NT + t + 1) * NQ],
                            start=(t == 0), stop=(t == NT - 1),
                        )
                for t in range(NT):
                    nc.tensor.matmul(
                        dn4_ps[:], draw[:, t * WAY:(t + 1) * WAY],
                        amat_t[:, t * NQ:(t + 1) * NQ],
                        start=(t == 0), stop=(t == NT - 1),
                    )
                osb = opool.tile([WAY, 2 * NQ], f32)
                nc.scalar.activation(osb[:, 0:NQ], dn4_ps[:], COPY)
                nc.scalar.activation(osb[:, NQ:2 * NQ], glob_ps[:], COPY)
                nc.sync.dma_start(out[e], osb[:])

            for e in range(EPC):
                qts = []
                for d in range(ND):
                    qt = qpool.tile([128, QP_PAD], bf16)
                    eng = nc.sync if d % 2 == 0 else nc.scalar
                    if e == 0:
                        eng.dma_start(qt[:, 0:256], qm[e, d, :, 0:256])
                    else:
                        eng.dma_start(qt[:], qm[e, d])
                    qts.append(qt)
                if e == 0:
                    for d in range(ND):
                        eng = nc.sync if d % 2 == 0 else nc.scalar
                        eng.dma_start(qts[d][:, 256:QP_PAD], qm[e, d, :, 256:QP_PAD])
                if e == 0:
                    # big constants ride behind the first episode's data
                    nc.sync.dma_start(amat_t[:], amat[:])
                    nc.scalar.dma_start(am2_t[:], am2[:])
                dn4_ps = accpool.tile([WAY, NQ], f32, tag="dn4ps")
                glob_ps = accpool.tile([WAY, NQ], f32, tag="globps")
                out8 = o8pool.tile([128, NT * WAY * 8], bf16)
                # tiles paired two-per-PSUM-bank: [0,1], [2,3], ..., [14]
                groups = [(2 * i, min(2 * i + 2, NT)) for i in range((NT + 1) // 2)]
                simsbs = {}
                for gi, (t0, t1) in enumerate(groups):
                    w = (t1 - t0) * SCOLS
                    simps = simpool.tile([128, 2 * SCOLS], f32, tag="simps")
                    for t in range(t0, t1):
                        off = (t - t0) * SCOLS
                        for d in range(ND):
                            nc.tensor.matmul(
                                simps[:, off:off + SCOLS],
                                qts[d][:, t * 128:(t + 1) * 128],
                                sts[d][:, e * SCOLS:(e + 1) * SCOLS],
                                start=(d == 0), stop=(d == ND - 1),
                            )
                    simsb = sbpool.tile([128, 2 * SCOLS], bf16)
                    for t in range(t0, t1):
                        off = (t - t0) * SCOLS
                        if gi == 0:
                            nc.vector.tensor_copy(
                                simsb[:, off:off + SCOLS], simps[:, off:off + SCOLS])
                        else:
                            nc.scalar.activation(
                                simsb[:, off:off + SCOLS], simps[:, off:off + SCOLS], COPY)
                        simsbs[t] = (simsb, off)
                        for kk in range(WAY):
                            g = t * WAY + kk
                            nc.vector.max(
                                out8[:, g * 8:(g + 1) * 8],
                                simsb[:, off + kk * HW:off + (kk + 1) * HW],
                            )
                    if gi == 1:
                        emit_tail()  # previous episode's aggregation matmuls
                    if e == EPC - 1:
                        for t in range(t0, t1):
                            simsb, off = simsbs[t]
                            nc.tensor.matmul(
                                glob_ps[:], simsb[:, off + WAY * HW:off + SCOLS],
                                am2_t[:, (e * NT + t) * NQ:(e * NT + t + 1) * NQ],
                                start=(t == 0), stop=(t == NT - 1),
                            )
                draw = drpool.tile([128, NT * WAY], bf16)
                o8v = out8[:].rearrange("p (g e) -> p g e", e=8)[:, :, 0:k]
                with nc.allow_low_precision("bf16 top-k sums feed a bf16 matmul"):
                    nc.vector.reduce_sum(draw[:], o8v, axis=mybir.AxisListType.X)
                pending.append((e, draw, dn4_ps, glob_ps, simsbs))
            emit_tail()
    nc.compile()
    return nc


def kernel(base, query, r, neighbor_k):
    from concourse.bass_utils import run_bass_kernel_spmd

    k = int(neighbor_k)
    assert 1 <= k <= 8, f"top-k must fit the Max8 output, got {k}"
    base = np.asarray(base, dtype=np.float32).reshape(B, WAY, D, HW)
    query = np.asarray(query, dtype=np.float32).reshape(B, NQ, D, HW)
    r = np.asarray(r, dtype=np.float32)

    # ---- host prep (layout + normalization metadata) ----
    # support: normalized columns + normalized class means -> s_ext [B, D, 130]
    s_norm = base / np.linalg.norm(base, axis=2, keepdims=True)
    bmean = base.mean(axis=3)                                     # [B, way, D]
    bm = bmean / np.maximum(
        np.linalg.norm(bmean, axis=2, keepdims=True), GEPS)
    s_ext = np.empty((B, D, SCOLS), dtype=np.float32)
    s_ext[:, :, :WAY * HW] = s_norm.transpose(0, 2, 1, 3).reshape(B, D, WAY * HW)
    s_ext[:, :, WAY * HW:] = bm.transpose(0, 2, 1)
    # [B, ND, 128, SCOLS] -> per-core [ND, 128, EPC*SCOLS]
    s_ext = s_ext.reshape(B, ND, 128, SCOLS).astype(ml_dtypes.bfloat16)
    s_ext = s_ext.reshape(N_CORES, EPC, ND, 128, SCOLS).transpose(0, 2, 3, 1, 4)
    s_ext = s_ext.reshape(N_CORES, ND, 128, EPC * SCOLS)

    # query: normalized patches, q_hat [B, D, 1920] (qp-major, zero-padded), bf16
    qn = np.sqrt(np.einsum("bqdp,bqdp->bqp", query, query))      # [B, nq, hw]
    q_hat = query / qn[:, :, None, :]
    q_mat = np.zeros((B, D, QP_PAD), dtype=ml_dtypes.bfloat16)
    q_mat[:, :, :QP] = q_hat.transpose(0, 2, 1, 3).reshape(B, D, QP)
    q_mat = q_mat.reshape(B, ND, 128, QP_PAD)
    qn_pad = np.zeros((B, QP_PAD), dtype=np.float32)
    qn_pad[:, :QP] = qn.reshape(B, QP)

    # query-mean norms for the global branch
    qmean = query.mean(axis=3)                                    # [B, nq, D]
    qmn = np.maximum(np.linalg.norm(qmean, axis=2), GEPS)         # [B, nq]

    # patch->query aggregation matrix (0/1), [128, NT*NQ]; and A*||q_patch||
    am = np.zeros((128, NT, NQ), dtype=np.float32)
    for t in range(NT):
        qp_idx = t * 128 + np.arange(128)
        valid = qp_idx < QP
        am[valid, t, qp_idx[valid] // HW] = 1.0
    am2 = am[None] * qn_pad.reshape(B, NT, 128).transpose(0, 2, 1)[:, :, :, None]
    am = am.reshape(128, NT * NQ).astype(ml_dtypes.bfloat16)
    am2 = am2.reshape(N_CORES, EPC, 128, NT * NQ).transpose(0, 2, 1, 3)
    am2 = np.ascontiguousarray(am2.reshape(N_CORES, 128, EPC * NT * NQ)).astype(ml_dtypes.bfloat16)

    if k not in _CACHE:
        _CACHE[k] = _build(k)
    nc = _CACHE[k]

    in_maps = []
    for c in range(N_CORES):
        sl = slice(c * EPC, (c + 1) * EPC)
        in_maps.append({
            "qm": np.ascontiguousarray(q_mat[sl]),
            "se": np.ascontiguousarray(s_ext[c]),
            "amat": am,
            "am2": am2[c],
        })
    global _LAST_IN_MAPS
    _LAST_IN_MAPS = in_maps
    res = run_bass_kernel_spmd(nc, in_maps, list(range(N_CORES)))
    dev = np.stack([res.results[c]["out"] for c in range(N_CORES)])  # [C, EPC, WAY, 150]
    dev = dev.reshape(B, WAY, 2 * NQ)

    dn4 = dev[:, :, :NQ].transpose(0, 2, 1) / k                   # [B, nq, way]
    glob = dev[:, :, NQ:].transpose(0, 2, 1) / (HW * qmn[:, :, None])
    return (r[0] * glob + r[1] * dn4).astype(np.float32)



# revision 7
# speedup vs baseline: 1.0850x; 1.0850x over previous
"""Trainium2 Bass kernel for nn_MetaBaseline (global-cosine + DN4 few-shot scoring).

Math (per episode b):
  global: logits[q,k] = <qmean_hat, bmean_hat>          (means over the 5x5 spatial grid)
  DN4:    sim[q,p,k,l] = <q_patch[q,p], s_col_hat[k,l]>  -> sum of top-neighbor_k over l,
          summed over p, / neighbor_k
  out = r0 * logits + r1 * dn4

Device strategy (data-parallel, 8 episodes per NeuronCore):
  - the global-cosine branch is 0.3% of the FLOPs and runs on the host
    (means + one small einsum); the device computes only the DN4 branch.
  - host pre-normalizes both sides: s_hat [640, 125] and q_hat laid out as
    q_mat [640, 1920] (qp-major, zero-padded from 1875) per episode, bf16.
  - PE: sim[qp, 0:125] = q_hat^T @ s_hat as 15 qp-tiles x 5 k-tiles of
    [128,128]x[128,125] bf16 matmuls; four qp-tiles share one fp32 PSUM bank
    [128,500] so one ACT copy covers four tiles.
  - DVE Max8 gives the top-8 of each 25-value support-patch group in one op;
    a strided reduce_sum of the first neighbor_k per episode gives the
    per-(patch,class) DN4 terms (on GpSimd, which is otherwise idle; the
    last episode's reduce runs on DVE to shorten the drain).
  - tiny matmuls against a 0/1 aggregation matrix contract the 25 patches of
    each query across partitions; the [5,75] result DMAs straight from PSUM.
  - startup: episode 0's inputs stream through small per-chunk SBUF tiles so
    the first matmul only waits on ~0.8MB, not the whole episode.
  - host applies 1/neighbor_k and the r-weighted combine.
"""
import numpy as np
import ml_dtypes

N_CORES = 8
B, WAY, SHOT, D, H, W = 64, 5, 1, 640, 5, 5
NQ = 75
HW = H * W                 # 25
QP = NQ * HW               # 1875 query patches per episode
NT = 15                    # qp tiles of 128
QP_PAD = NT * 128          # 1920
ND = D // 128              # 5 contraction tiles
EPC = B // N_CORES         # 8 episodes per core
SCOLS = WAY * HW           # 125
GEPS = 1e-12               # eps of the global-cosine branch (torch F.normalize)

# episode-0 q column chunks (tile ranges) for precise startup deps
E0_CHUNKS = [(0, 512), (512, 1280), (1280, 1920)]

_CACHE = {}
_LAST_IN_MAPS = None


def _build(k: int):
    """Build + compile the SPMD NEFF for top-k = k (k <= 8)."""
    import concourse.bacc as bacc
    import concourse.mybir as mybir
    import concourse.tile as tile

    bf16 = mybir.dt.bfloat16
    f32 = mybir.dt.float32
    COPY = mybir.ActivationFunctionType.Copy

    nc = bacc.Bacc("TRN2", target_bir_lowering=False, debug=False)
    qm = nc.dram_tensor("qm", [EPC, ND, 128, QP_PAD], bf16, kind="ExternalInput")
    se = nc.dram_tensor("se", [ND, 128, EPC * SCOLS], bf16, kind="ExternalInput")
    amat = nc.dram_tensor("amat", [128, NT * NQ], bf16, kind="ExternalInput")
    out = nc.dram_tensor("out", [EPC, WAY, NQ], f32, kind="ExternalOutput")

    with tile.TileContext(nc) as tc:
        with (
            tc.tile_pool(name="const", bufs=1) as cpool,
            tc.tile_pool(name="qe0", bufs=1) as e0pool,
            tc.tile_pool(name="q", bufs=4 * ND) as qpool,
            tc.tile_pool(name="simps", bufs=4, space="PSUM") as simpool,
            tc.tile_pool(name="acc", bufs=2, space="PSUM") as accpool,
            tc.tile_pool(name="simsb", bufs=6) as sbpool,
            tc.tile_pool(name="out8", bufs=3) as o8pool,
            tc.tile_pool(name="draw", bufs=3) as drpool,
            tc.tile_pool(name="osb", bufs=2) as opool,
        ):
            # episode 0 support columns first (tiny), rest later
            se0, ser = [], []
            for d in range(ND):
                st0 = cpool.tile([128, SCOLS], bf16, tag=f"se0_{d}")
                (nc.sync if d % 2 == 0 else nc.gpsimd).dma_start(
                    st0[:], se[d, :, 0:SCOLS])
                se0.append(st0)
            amat_t = cpool.tile([128, NT * NQ], bf16)

            pending = []  # deferred tail: (e, draw)

            def emit_tail():
                if not pending:
                    return
                e, draw = pending.pop()
                dn4_ps = accpool.tile([WAY, NQ], f32, tag="dn4ps")
                for t in range(NT):
                    nc.tensor.matmul(
                        dn4_ps[:], draw[:, t * WAY:(t + 1) * WAY],
                        amat_t[:, t * NQ:(t + 1) * NQ],
                        start=(t == 0), stop=(t == NT - 1),
                    )
                osb = opool.tile([WAY, NQ], f32)
                nc.scalar.activation(osb[:], dn4_ps[:], COPY)
                nc.scalar.dma_start(out[e], osb[:])

            for e in range(EPC):
                if e == 0:
                    # per-chunk tiles: the first matmuls wait only on chunk a
                    qe0 = []
                    for c0, c1 in E0_CHUNKS:
                        row = []
                        for d in range(ND):
                            qt = e0pool.tile([128, c1 - c0], bf16,
                                             tag=f"qe0_{c0}_{d}")
                            eng = nc.sync if d % 2 == 0 else nc.gpsimd
                            eng.dma_start(qt[:], qm[e, d, :, c0:c1])
                            row.append(qt)
                        qe0.append(row)
                    # constants + remaining support ride behind episode 0
                    nc.scalar.dma_start(amat_t[:], amat[:])
                    for d in range(ND):
                        st = cpool.tile([128, (EPC - 1) * SCOLS], bf16,
                                        tag=f"ser_{d}")
                        (nc.sync if d % 2 == 0 else nc.gpsimd).dma_start(
                            st[:], se[d, :, SCOLS:EPC * SCOLS])
                        ser.append(st)

                    def qsl(t, d):
                        for ci, (c0, c1) in enumerate(E0_CHUNKS):
                            if t * 128 < c1:
                                return qe0[ci][d][:, t * 128 - c0:t * 128 - c0 + 128]
                        raise AssertionError

                    def ssl(d):
                        return se0[d][:]
                else:
                    qts = []
                    for d in range(ND):
                        qt = qpool.tile([128, QP_PAD], bf16)
                        eng = nc.sync if d % 2 == 0 else nc.gpsimd
                        eng.dma_start(qt[:], qm[e, d])
                        qts.append(qt)

                    def qsl(t, d):
                        return qts[d][:, t * 128:(t + 1) * 128]

                    def ssl(d):
                        return ser[d][:, (e - 1) * SCOLS:e * SCOLS]

                out8 = o8pool.tile([128, NT * WAY * 8], bf16)
                # four tiles share one PSUM bank: 4*125 fp32 = 2000B
                groups = [(0, 4), (4, 8), (8, 12), (12, 15)]
                for gi, (t0, t1) in enumerate(groups):
                    w = (t1 - t0) * SCOLS
                    simps = simpool.tile([128, 500], f32, tag="simps")
                    for t in range(t0, t1):
                        off = (t - t0) * SCOLS
                        for d in range(ND):
                            nc.tensor.matmul(
                                simps[:, off:off + SCOLS],
                                qsl(t, d), ssl(d),
                                start=(d == 0), stop=(d == ND - 1),
                            )
                    simsb = sbpool.tile([128, 500], bf16)
                    nc.scalar.activation(simsb[:, 0:w], simps[:, 0:w], COPY)
                    for t in range(t0, t1):
                        off = (t - t0) * SCOLS
                        for kk in range(WAY):
                            g = t * WAY + kk
                            nc.vector.max(
                                out8[:, g * 8:(g + 1) * 8],
                                simsb[:, off + kk * HW:off + (kk + 1) * HW],
                            )
                    if gi == 1:
                        emit_tail()  # previous episode's aggregation matmuls
                draw = drpool.tile([128, NT * WAY], bf16)
                o8 = out8[:].rearrange("p (g e) -> p g e", e=8)
                with nc.allow_low_precision("bf16 top-k sums feed a bf16 matmul"):
                    if e == EPC - 1:
                        # drain fast on DVE (free-axis reduce is DVE-only)
                        nc.vector.reduce_sum(
                            draw[:], o8[:, :, 0:k], axis=mybir.AxisListType.X)
                    elif k == 1:
                        nc.gpsimd.tensor_copy(draw[:], o8[:, :, 0])
                    else:
                        # GpSimd lacks free-axis reduce; strided add chain
                        nc.gpsimd.tensor_add(draw[:], o8[:, :, 0], o8[:, :, 1])
                        for j in range(2, k):
                            nc.gpsimd.tensor_add(draw[:], draw[:], o8[:, :, j])
                pending.append((e, draw))
            emit_tail()
    nc.compile()
    return nc


def kernel(base, query, r, neighbor_k):
    from concourse.bass_utils import run_bass_kernel_spmd

    k = int(neighbor_k)
    assert 1 <= k <= 8, f"top-k must fit the Max8 output, got {k}"
    base = np.asarray(base, dtype=np.float32).reshape(B, WAY, D, HW)
    query = np.asarray(query, dtype=np.float32).reshape(B, NQ, D, HW)
    r = np.asarray(r, dtype=np.float32)

    # ---- host prep (layout + normalization) ----
    # support: normalized columns -> s_hat [B, D, 125]
    s_norm = base / np.linalg.norm(base, axis=2, keepdims=True)
    s_ext = s_norm.transpose(0, 2, 1, 3).reshape(B, D, SCOLS)
    s_ext = s_ext.reshape(B, ND, 128, SCOLS).astype(ml_dtypes.bfloat16)
    s_ext = s_ext.reshape(N_CORES, EPC, ND, 128, SCOLS).transpose(0, 2, 3, 1, 4)
    s_ext = s_ext.reshape(N_CORES, ND, 128, EPC * SCOLS)

    # query: normalized patches, q_hat [B, D, 1920] (qp-major, zero-padded), bf16
    qn = np.sqrt(np.einsum("bqdp,bqdp->bqp", query, query))      # [B, nq, hw]
    q_hat = query / qn[:, :, None, :]
    q_mat = np.zeros((B, D, QP_PAD), dtype=ml_dtypes.bfloat16)
    q_mat[:, :, :QP] = q_hat.transpose(0, 2, 1, 3).reshape(B, D, QP)
    q_mat = q_mat.reshape(B, ND, 128, QP_PAD)

    # global-cosine branch on host (0.3% of the FLOPs)
    bmean = base.mean(axis=3)                                     # [B, way, D]
    bm = bmean / np.maximum(
        np.linalg.norm(bmean, axis=2, keepdims=True), GEPS)
    qmean = query.mean(axis=3)                                    # [B, nq, D]
    qm_hat = qmean / np.maximum(
        np.linalg.norm(qmean, axis=2, keepdims=True), GEPS)
    glob = np.einsum("bqd,bkd->bqk", qm_hat, bm)                  # [B, nq, way]

    # patch->query aggregation matrix (0/1), [128, NT*NQ]
    am = np.zeros((128, NT, NQ), dtype=np.float32)
    for t in range(NT):
        qp_idx = t * 128 + np.arange(128)
        valid = qp_idx < QP
        am[valid, t, qp_idx[valid] // HW] = 1.0
    am = am.reshape(128, NT * NQ).astype(ml_dtypes.bfloat16)

    if k not in _CACHE:
        _CACHE[k] = _build(k)
    nc = _CACHE[k]

    in_maps = []
    for c in range(N_CORES):
        sl = slice(c * EPC, (c + 1) * EPC)
        in_maps.append({
            "qm": np.ascontiguousarray(q_mat[sl]),
            "se": np.ascontiguousarray(s_ext[c]),
            "amat": am,
        })
    global _LAST_IN_MAPS
    _LAST_IN_MAPS = in_maps
    res = run_bass_kernel_spmd(nc, in_maps, list(range(N_CORES)))
    dev = np.stack([res.results[c]["out"] for c in range(N_CORES)])  # [C, EPC, WAY, NQ]
    dn4 = dev.reshape(B, WAY, NQ).transpose(0, 2, 1) / k             # [B, nq, way]
    return (r[0] * glob + r[1] * dn4).astype(np.float32)


# revision 8
# speedup vs baseline: 1.3198x; 1.2164x over previous
"""Trainium2 Bass kernel for nn_MetaBaseline (global-cosine + DN4 few-shot scoring).

Math (per episode b):
  global: logits[q,k] = <qmean_hat, bmean_hat>          (means over the 5x5 spatial grid)
  DN4:    sim[q,p,k,l] = <q_patch[q,p], s_col_hat[k,l]>  -> sum of top-neighbor_k over l,
          summed over p, / neighbor_k
  out = r0 * logits + r1 * dn4

Device strategy (data-parallel, 8 episodes per NeuronCore):
  - the global-cosine branch is 0.3% of the FLOPs and runs on the host
    (means + one small einsum); the device computes only the DN4 branch.
  - host pre-normalizes both sides: s_hat [640, 125] bf16 and q_hat laid out
    as q_mat [640, 1920] (qp-major, zero-padded from 1875) per episode in
    fp8 e4m3 scaled by 16 (halves the dominant HBM stream; the x16 rides
    through the whole DN4 branch and is divided out on the host).
  - PE: sim[qp, 0:125] = q_hat^T @ s_hat as 15 qp-tiles x 5 k-tiles of
    [128,128]x[128,125] fp8xbf16 matmuls; four qp-tiles share one fp32 PSUM
    bank [128,500] so one ACT copy covers four tiles.  A short burst of
    dummy matmuls warms the PE clock up while episode 0 streams in.
  - DVE Max8 gives the top-8 of each 25-value support-patch group in one op;
    a strided add-chain of the first neighbor_k per episode gives the
    per-(patch,class) DN4 terms (on GpSimd, which is otherwise idle; the
    last episode reduces per-group on DVE so the drain is short).
  - tiny matmuls against a 0/1 aggregation matrix contract the 25 patches of
    each query across partitions.
  - startup: episode 0's first columns and support ride in two merged "head"
    DMAs so the first matmul waits on ~0.3MB, not the whole episode.
  - host applies 1/(16*neighbor_k) and the r-weighted combine.
"""
import numpy as np
import ml_dtypes

N_CORES = 8
B, WAY, SHOT, D, H, W = 64, 5, 1, 640, 5, 5
NQ = 75
HW = H * W                 # 25
QP = NQ * HW               # 1875 query patches per episode
NT = 15                    # qp tiles of 128
QP_PAD = NT * 128          # 1920
ND = D // 128              # 5 contraction tiles
EPC = B // N_CORES         # 8 episodes per core
SCOLS = WAY * HW           # 125
GEPS = 1e-12               # eps of the global-cosine branch (torch F.normalize)
QSCALE = 16.0              # fp8 pre-scale for q_hat
HEAD = 512                 # episode-0 head columns (tiles 0-3)
N_WARM = 24                # PE clock warm-up matmuls

_CACHE = {}
_LAST_IN_MAPS = None


def _build(k: int):
    """Build + compile the SPMD NEFF for top-k = k (k <= 8)."""
    import concourse.bacc as bacc
    import concourse.mybir as mybir
    import concourse.tile as tile

    bf16 = mybir.dt.bfloat16
    fp8 = mybir.dt.float8e4
    f32 = mybir.dt.float32
    COPY = mybir.ActivationFunctionType.Copy

    nc = bacc.Bacc("TRN2", target_bir_lowering=False, debug=False)
    qm = nc.dram_tensor("qm", [EPC, ND, 128, QP_PAD], fp8, kind="ExternalInput")
    q0h = nc.dram_tensor("q0h", [128, ND * HEAD], fp8, kind="ExternalInput")
    se = nc.dram_tensor("se", [ND, 128, EPC * SCOLS], bf16, kind="ExternalInput")
    se0h = nc.dram_tensor("se0h", [128, ND * SCOLS], bf16, kind="ExternalInput")
    amat = nc.dram_tensor("amat", [128, NT * NQ], bf16, kind="ExternalInput")
    out = nc.dram_tensor("out", [EPC, WAY, NQ], f32, kind="ExternalOutput")

    with tile.TileContext(nc) as tc:
        with (
            tc.tile_pool(name="const", bufs=1) as cpool,
            tc.tile_pool(name="qe0", bufs=1) as e0pool,
            tc.tile_pool(name="q", bufs=4 * ND) as qpool,
            tc.tile_pool(name="warm", bufs=1) as wpool,
            tc.tile_pool(name="warmps", bufs=1, space="PSUM") as wpspool,
            tc.tile_pool(name="simps", bufs=4, space="PSUM") as simpool,
            tc.tile_pool(name="acc", bufs=2, space="PSUM") as accpool,
            tc.tile_pool(name="simsb", bufs=6) as sbpool,
            tc.tile_pool(name="out8", bufs=3) as o8pool,
            tc.tile_pool(name="draw", bufs=3) as drpool,
            tc.tile_pool(name="osb", bufs=2) as opool,
        ):
            # merged heads: first matmul waits on just these two DMAs
            se0h_t = cpool.tile([128, ND * SCOLS], bf16)
            nc.sync.dma_start(se0h_t[:], se0h[:])
            q0h_t = cpool.tile([128, ND * HEAD], fp8)
            nc.gpsimd.dma_start(q0h_t[:], q0h[:])

            # PE clock warm-up on garbage-free data while episode 0 streams
            wt = wpool.tile([128, 128], bf16)
            nc.gpsimd.memset(wt[:], 0.0)
            wps = wpspool.tile([128, 128], f32)
            for _ in range(N_WARM):
                nc.tensor.matmul(wps[:], wt[:], wt[:], start=True, stop=True)

            pending = []  # deferred tail: (e, draw)

            def emit_tail():
                if not pending:
                    return
                e, draw = pending.pop()
                dn4_ps = accpool.tile([WAY, NQ], f32, tag="dn4ps")
                for t in range(NT):
                    nc.tensor.matmul(
                        dn4_ps[:], draw[:, t * WAY:(t + 1) * WAY],
                        amat_t[:, t * NQ:(t + 1) * NQ],
                        start=(t == 0), stop=(t == NT - 1),
                    )
                osb = opool.tile([WAY, NQ], f32)
                nc.scalar.activation(osb[:], dn4_ps[:], COPY)
                nc.scalar.dma_start(out[e], osb[:])

            ser = []
            for e in range(EPC):
                if e == 0:
                    # rest of episode 0 in two per-chunk tiles per d
                    qe0 = {}
                    for ci, (c0, c1) in enumerate([(HEAD, 1280), (1280, 1920)]):
                        for d in range(ND):
                            qt = e0pool.tile([128, c1 - c0], fp8,
                                             tag=f"qe0_{c0}_{d}")
                            eng = nc.sync if (ci * ND + d) % 2 == 0 else nc.gpsimd
                            eng.dma_start(qt[:], qm[e, d, :, c0:c1])
                            qe0[(ci, d)] = qt
                    # constants + remaining support ride behind episode 0
                    amat_t = cpool.tile([128, NT * NQ], bf16)
                    nc.scalar.dma_start(amat_t[:], amat[:])
                    for d in range(ND):
                        st = cpool.tile([128, (EPC - 1) * SCOLS], bf16,
                                        tag=f"ser_{d}")
                        (nc.sync if d % 2 == 0 else nc.gpsimd).dma_start(
                            st[:], se[d, :, SCOLS:EPC * SCOLS])
                        ser.append(st)

                    def qsl(t, d):
                        c = t * 128
                        if c < HEAD:
                            return q0h_t[:, d * HEAD + c:d * HEAD + c + 128]
                        ci, c0 = (0, HEAD) if c < 1280 else (1, 1280)
                        return qe0[(ci, d)][:, c - c0:c - c0 + 128]

                    def ssl(d):
                        return se0h_t[:, d * SCOLS:(d + 1) * SCOLS]
                else:
                    qts = []
                    for d in range(ND):
                        qt = qpool.tile([128, QP_PAD], fp8)
                        eng = nc.sync if d % 2 == 0 else nc.gpsimd
                        eng.dma_start(qt[:], qm[e, d])
                        qts.append(qt)

                    def qsl(t, d):
                        return qts[d][:, t * 128:(t + 1) * 128]

                    def ssl(d):
                        return ser[d][:, (e - 1) * SCOLS:e * SCOLS]

                last = e == EPC - 1
                out8 = o8pool.tile([128, NT * WAY * 8], bf16)
                o8 = out8[:].rearrange("p (g e) -> p g e", e=8)
                draw = drpool.tile([128, NT * WAY], bf16)
                if last:
                    dn4_ps = accpool.tile([WAY, NQ], f32, tag="dn4ps")
                # four tiles share one PSUM bank: 4*125 fp32 = 2000B
                groups = [(0, 4), (4, 8), (8, 12), (12, 15)]
                for gi, (t0, t1) in enumerate(groups):
                    w = (t1 - t0) * SCOLS
                    simps = simpool.tile([128, 500], f32, tag="simps")
                    for t in range(t0, t1):
                        off = (t - t0) * SCOLS
                        for d in range(ND):
                            nc.tensor.matmul(
                                simps[:, off:off + SCOLS],
                                qsl(t, d), ssl(d),
                                start=(d == 0), stop=(d == ND - 1),
                            )
                    simsb = sbpool.tile([128, 500], bf16)
                    nc.scalar.activation(simsb[:, 0:w], simps[:, 0:w], COPY)
                    for t in range(t0, t1):
                        off = (t - t0) * SCOLS
                        for kk in range(WAY):
                            g = t * WAY + kk
                            nc.vector.max(
                                out8[:, g * 8:(g + 1) * 8],
                                simsb[:, off + kk * HW:off + (kk + 1) * HW],
                            )
                    if last:
                        # reduce + aggregate this group now: short drain
                        g0, g1 = t0 * WAY, t1 * WAY
                        with nc.allow_low_precision("bf16 top-k sums"):
                            nc.vector.reduce_sum(
                                draw[:, g0:g1], o8[:, g0:g1, 0:k],
                                axis=mybir.AxisListType.X)
                        for t in range(t0, t1):
                            nc.tensor.matmul(
                                dn4_ps[:], draw[:, t * WAY:(t + 1) * WAY],
                                amat_t[:, t * NQ:(t + 1) * NQ],
                                start=(t == 0), stop=(t == NT - 1),
                            )
                    if gi == 1:
                        emit_tail()  # previous episode's aggregation matmuls
                if last:
                    osb = opool.tile([WAY, NQ], f32)
                    nc.scalar.activation(osb[:], dn4_ps[:], COPY)
                    nc.scalar.dma_start(out[e], osb[:])
                else:
                    with nc.allow_low_precision("bf16 top-k sums"):
                        if k == 1:
                            nc.gpsimd.tensor_copy(draw[:], o8[:, :, 0])
                        else:
                            # GpSimd lacks free-axis reduce; strided add chain
                            nc.gpsimd.tensor_add(
                                draw[:], o8[:, :, 0], o8[:, :, 1])
                            for j in range(2, k):
                                nc.gpsimd.tensor_add(
                                    draw[:], draw[:], o8[:, :, j])
                    pending.append((e, draw))
            emit_tail()
    nc.compile()
    return nc


def kernel(base, query, r, neighbor_k):
    from concourse.bass_utils import run_bass_kernel_spmd

    k = int(neighbor_k)
    assert 1 <= k <= 8, f"top-k must fit the Max8 output, got {k}"
    base = np.asarray(base, dtype=np.float32).reshape(B, WAY, D, HW)
    query = np.asarray(query, dtype=np.float32).reshape(B, NQ, D, HW)
    r = np.asarray(r, dtype=np.float32)

    # ---- host prep (layout + normalization) ----
    # support: normalized columns -> s_hat [B, D, 125]
    s_norm = base / np.linalg.norm(base, axis=2, keepdims=True)
    s_ext = s_norm.transpose(0, 2, 1, 3).reshape(B, D, SCOLS)
    s_ext = s_ext.reshape(B, ND, 128, SCOLS).astype(ml_dtypes.bfloat16)
    s_ext = s_ext.reshape(N_CORES, EPC, ND, 128, SCOLS).transpose(0, 2, 3, 1, 4)
    s_ext = s_ext.reshape(N_CORES, ND, 128, EPC * SCOLS)

    # query: normalized patches scaled x16 in fp8, [B, D, 1920] qp-major
    qn = np.sqrt(np.einsum("bqdp,bqdp->bqp", query, query))      # [B, nq, hw]
    q_hat = query * (QSCALE / qn[:, :, None, :])
    q_mat = np.zeros((B, D, QP_PAD), dtype=ml_dtypes.float8_e4m3)
    q_mat[:, :, :QP] = q_hat.transpose(0, 2, 1, 3).reshape(B, D, QP)
    q_mat = q_mat.reshape(B, ND, 128, QP_PAD)

    # global-cosine branch on host (0.3% of the FLOPs)
    bmean = base.mean(axis=3)                                     # [B, way, D]
    bm = bmean / np.maximum(
        np.linalg.norm(bmean, axis=2, keepdims=True), GEPS)
    qmean = query.mean(axis=3)                                    # [B, nq, D]
    qm_hat = qmean / np.maximum(
        np.linalg.norm(qmean, axis=2, keepdims=True), GEPS)
    glob = np.einsum("bqd,bkd->bqk", qm_hat, bm)                  # [B, nq, way]

    # patch->query aggregation matrix (0/1), [128, NT*NQ]
    am = np.zeros((128, NT, NQ), dtype=np.float32)
    for t in range(NT):
        qp_idx = t * 128 + np.arange(128)
        valid = qp_idx < QP
        am[valid, t, qp_idx[valid] // HW] = 1.0
    am = am.reshape(128, NT * NQ).astype(ml_dtypes.bfloat16)

    if k not in _CACHE:
        _CACHE[k] = _build(k)
    nc = _CACHE[k]

    in_maps = []
    for c in range(N_CORES):
        sl = slice(c * EPC, (c + 1) * EPC)
        qc = np.ascontiguousarray(q_mat[sl])                      # [EPC,ND,128,1920]
        sc = s_ext[c]                                             # [ND,128,EPC*125]
        in_maps.append({
            "qm": qc,
            "q0h": np.ascontiguousarray(
                qc[0, :, :, :HEAD].transpose(1, 0, 2).reshape(128, ND * HEAD)),
            "se": np.ascontiguousarray(sc),
            "se0h": np.ascontiguousarray(
                sc[:, :, :SCOLS].transpose(1, 0, 2).reshape(128, ND * SCOLS)),
            "amat": am,
        })
    global _LAST_IN_MAPS
    _LAST_IN_MAPS = in_maps
    res = run_bass_kernel_spmd(nc, in_maps, list(range(N_CORES)))
    dev = np.stack([res.results[c]["out"] for c in range(N_CORES)])  # [C, EPC, WAY, NQ]
    dn4 = dev.reshape(B, WAY, NQ).transpose(0, 2, 1) / (QSCALE * k)  # [B, nq, way]
    return (r[0] * glob + r[1] * dn4).astype(np.float32)


# revision 11
# speedup vs baseline: 1.3581x; 1.0290x over previous
"""Trainium2 Bass kernel for nn_MetaBaseline (global-cosine + DN4 few-shot scoring).

Math (per episode b):
  global: logits[q,k] = <qmean_hat, bmean_hat>          (means over the 5x5 spatial grid)
  DN4:    sim[q,p,k,l] = <q_patch[q,p], s_col_hat[k,l]>  -> sum of top-neighbor_k over l,
          summed over p, / neighbor_k
  out = r0 * logits + r1 * dn4

Device strategy (data-parallel, 8 episodes per NeuronCore):
  - the global-cosine branch is 0.3% of the FLOPs and runs on the host
    (means + one small einsum); the device computes only the DN4 branch.
  - host pre-normalizes both sides: s_hat packed [128, nd*125] bf16 per
    episode and q_hat packed [128, nd*1920] (qp-major, zero-padded from 1875)
    per episode in fp8 e4m3 scaled by 16 (halves the dominant HBM stream and
    makes each episode one contiguous 9600B-per-partition DMA; the x16 rides
    through the whole DN4 branch and is divided out on the host).
  - PE: sim[qp, 0:125] = q_hat^T @ s_hat as 15 qp-tiles x 5 k-tiles of
    [128,128]x[128,125] fp8xbf16 matmuls; four qp-tiles share one fp32 PSUM
    bank [128,500] so one ACT copy covers four tiles; 6 sim banks let the PE
    run ~1.5 episodes ahead of DVE while its clock ramps up.
  - DVE Max8 gives the top-8 of each 25-value support-patch group in one op;
    a strided add-chain of the first neighbor_k per episode gives the
    per-(patch,class) DN4 terms (on GpSimd, which is otherwise idle; the
    last episode reduces per-group on DVE so the drain is short).
  - tiny matmuls against a 0/1 aggregation matrix contract the 25 patches of
    each query across partitions.
  - startup: episode 0 streams in three column chunks so the first matmul
    waits on ~0.5MB; later episodes prefetch 4 deep on two DMA queues.
  - host applies 1/(16*neighbor_k) and the r-weighted combine.
"""
import numpy as np
import ml_dtypes

N_CORES = 8
B, WAY, SHOT, D, H, W = 64, 5, 1, 640, 5, 5
NQ = 75
HW = H * W                 # 25
QP = NQ * HW               # 1875 query patches per episode
NT = 15                    # qp tiles of 128
QP_PAD = NT * 128          # 1920
ND = D // 128              # 5 contraction tiles
EPC = B // N_CORES         # 8 episodes per core
SCOLS = WAY * HW           # 125
GEPS = 1e-12               # eps of the global-cosine branch (torch F.normalize)
QSCALE = 16.0              # fp8 pre-scale for q_hat
HEAD = 512                 # episode-0 head columns (tiles 0-3)
C1 = 1280                  # episode-0 second chunk boundary

_CACHE = {}
_LAST_IN_MAPS = None


def _build(k: int):
    """Build + compile the SPMD NEFF for top-k = k (k <= 8)."""
    import concourse.bacc as bacc
    import concourse.mybir as mybir
    import concourse.tile as tile

    bf16 = mybir.dt.bfloat16
    fp8 = mybir.dt.float8e4
    f32 = mybir.dt.float32
    COPY = mybir.ActivationFunctionType.Copy

    nc = bacc.Bacc("TRN2", target_bir_lowering=False, debug=False)
    # per-episode packed layouts: one contiguous DMA per episode
    qp8 = nc.dram_tensor("qp8", [EPC, 128, ND * QP_PAD], fp8, kind="ExternalInput")
    q0h = nc.dram_tensor("q0h", [128, ND * HEAD], fp8, kind="ExternalInput")
    seh = nc.dram_tensor("seh", [EPC, 128, ND * SCOLS], bf16, kind="ExternalInput")
    amat = nc.dram_tensor("amat", [128, NT * NQ], bf16, kind="ExternalInput")
    out = nc.dram_tensor("out", [EPC, WAY, NQ], f32, kind="ExternalOutput")

    qv = qp8[:].rearrange("e p (d c) -> e p d c", d=ND)  # strided e0 chunk views

    with tile.TileContext(nc) as tc:
        with (
            tc.tile_pool(name="const", bufs=1) as cpool,
            tc.tile_pool(name="qe0", bufs=1) as e0pool,
            tc.tile_pool(name="q", bufs=4) as qpool,
            tc.tile_pool(name="simps", bufs=6, space="PSUM") as simpool,
            tc.tile_pool(name="acc", bufs=2, space="PSUM") as accpool,
            tc.tile_pool(name="simsb", bufs=8) as sbpool,
            tc.tile_pool(name="out8", bufs=3) as o8pool,
            tc.tile_pool(name="draw", bufs=3) as drpool,
            tc.tile_pool(name="osb", bufs=2) as opool,
        ):
            # heads: the first matmul waits on just these two DMAs
            seh_t = {0: cpool.tile([128, ND * SCOLS], bf16, tag="seh0",
                                   name="seh0")}
            nc.sync.dma_start(seh_t[0][:], seh[0])
            q0h_t = cpool.tile([128, ND * HEAD], fp8)
            nc.gpsimd.dma_start(q0h_t[:], q0h[:])
            # rest of episode 0 in two packed-chunk tiles (strided DRAM view)
            qe0b = e0pool.tile([128, ND * (C1 - HEAD)], fp8)
            nc.sync.dma_start(
                qe0b[:].rearrange("p (d c) -> p d c", d=ND),
                qv[0, :, :, HEAD:C1])
            qe0c = e0pool.tile([128, ND * (QP_PAD - C1)], fp8)
            nc.gpsimd.dma_start(
                qe0c[:].rearrange("p (d c) -> p d c", d=ND),
                qv[0, :, :, C1:QP_PAD])

            pending = []  # deferred tail: (e, draw)

            def emit_tail():
                if not pending:
                    return
                e, draw = pending.pop()
                dn4_ps = accpool.tile([WAY, NQ], f32, tag="dn4ps")
                for t in range(NT):
                    nc.tensor.matmul(
                        dn4_ps[:], draw[:, t * WAY:(t + 1) * WAY],
                        amat_t[:, t * NQ:(t + 1) * NQ],
                        start=(t == 0), stop=(t == NT - 1),
                    )
                osb = opool.tile([WAY, NQ], f32)
                nc.scalar.activation(osb[:], dn4_ps[:], COPY)
                nc.scalar.dma_start(out[e], osb[:])

            qts = {}
            for e in range(EPC):
                if e == 0:
                    def qsl(t, d):
                        c = t * 128
                        if c < HEAD:
                            return q0h_t[:, d * HEAD + c:d * HEAD + c + 128]
                        if c < C1:
                            w = C1 - HEAD
                            return qe0b[:, d * w + c - HEAD:d * w + c - HEAD + 128]
                        w = QP_PAD - C1
                        return qe0c[:, d * w + c - C1:d * w + c - C1 + 128]
                else:
                    # prefetch: se slice + one contiguous q DMA per episode
                    eng = nc.sync if e % 2 == 1 else nc.gpsimd
                    seh_t[e] = cpool.tile([128, ND * SCOLS], bf16,
                                          tag=f"seh{e}", name=f"seh{e}")
                    eng.dma_start(seh_t[e][:], seh[e])
                    qt = qpool.tile([128, ND * QP_PAD], fp8)
                    eng.dma_start(qt[:], qp8[e])
                    qts[e] = qt
                    if e == 1:
                        amat_t = cpool.tile([128, NT * NQ], bf16)
                        nc.scalar.dma_start(amat_t[:], amat[:])

                    def qsl(t, d, qt=qt):
                        return qt[:, d * QP_PAD + t * 128:d * QP_PAD + t * 128 + 128]

                def ssl(d, e=e):
                    return seh_t[e][:, d * SCOLS:(d + 1) * SCOLS]

                last = e == EPC - 1
                out8 = o8pool.tile([128, NT * WAY * 8], bf16)
                o8 = out8[:].rearrange("p (g e) -> p g e", e=8)
                draw = drpool.tile([128, NT * WAY], bf16)
                if last:
                    dn4_ps = accpool.tile([WAY, NQ], f32, tag="dn4ps")
                # four tiles share one PSUM bank: 4*125 fp32 = 2000B
                groups = [(0, 4), (4, 8), (8, 12), (12, 15)]
                for gi, (t0, t1) in enumerate(groups):
                    w = (t1 - t0) * SCOLS
                    simps = simpool.tile([128, 500], f32, tag="simps")
                    for t in range(t0, t1):
                        off = (t - t0) * SCOLS
                        for d in range(ND):
                            nc.tensor.matmul(
                                simps[:, off:off + SCOLS],
                                qsl(t, d), ssl(d),
                                start=(d == 0), stop=(d == ND - 1),
                            )
                    simsb = sbpool.tile([128, 500], bf16)
                    nc.scalar.activation(simsb[:, 0:w], simps[:, 0:w], COPY)
                    for t in range(t0, t1):
                        off = (t - t0) * SCOLS
                        for kk in range(WAY):
                            g = t * WAY + kk
                            nc.vector.max(
                                out8[:, g * 8:(g + 1) * 8],
                                simsb[:, off + kk * HW:off + (kk + 1) * HW],
                            )
                    if last:
                        # reduce + aggregate this group now: short drain
                        g0, g1 = t0 * WAY, t1 * WAY
                        with nc.allow_low_precision("bf16 top-k sums"):
                            nc.vector.reduce_sum(
                                draw[:, g0:g1], o8[:, g0:g1, 0:k],
                                axis=mybir.AxisListType.X)
                        for t in range(t0, t1):
                            nc.tensor.matmul(
                                dn4_ps[:], draw[:, t * WAY:(t + 1) * WAY],
                                amat_t[:, t * NQ:(t + 1) * NQ],
                                start=(t == 0), stop=(t == NT - 1),
                            )
                    if gi == 1:
                        emit_tail()  # previous episode's aggregation matmuls
                if last:
                    osb = opool.tile([WAY, NQ], f32)
                    nc.scalar.activation(osb[:], dn4_ps[:], COPY)
                    nc.scalar.dma_start(out[e], osb[:])
                else:
                    with nc.allow_low_precision("bf16 top-k sums"):
                        if k == 1:
                            nc.gpsimd.tensor_copy(draw[:], o8[:, :, 0])
                        else:
                            # GpSimd lacks free-axis reduce; strided add chain
                            nc.gpsimd.tensor_add(
                                draw[:], o8[:, :, 0], o8[:, :, 1])
                            for j in range(2, k):
                                nc.gpsimd.tensor_add(
                                    draw[:], draw[:], o8[:, :, j])
                    pending.append((e, draw))
            emit_tail()
    nc.compile()
    return nc


def kernel(base, query, r, neighbor_k):
    from concourse.bass_utils import run_bass_kernel_spmd

    k = int(neighbor_k)
    assert 1 <= k <= 8, f"top-k must fit the Max8 output, got {k}"
    base = np.asarray(base, dtype=np.float32).reshape(B, WAY, D, HW)
    query = np.asarray(query, dtype=np.float32).reshape(B, NQ, D, HW)
    r = np.asarray(r, dtype=np.float32)

    # ---- host prep (layout + normalization) ----
    # support: normalized columns packed per episode -> [B, 128, ND*125] bf16
    s_norm = base / np.linalg.norm(base, axis=2, keepdims=True)
    s_ext = s_norm.transpose(0, 2, 1, 3).reshape(B, ND, 128, SCOLS)
    seh = np.ascontiguousarray(s_ext.transpose(0, 2, 1, 3)).reshape(
        B, 128, ND * SCOLS).astype(ml_dtypes.bfloat16)

    # query: normalized patches scaled x16 in fp8, packed [B, 128, ND*1920]
    qn = np.sqrt(np.einsum("bqdp,bqdp->bqp", query, query))      # [B, nq, hw]
    q_hat = query * (QSCALE / qn[:, :, None, :])
    q_mat = np.zeros((B, D, QP_PAD), dtype=ml_dtypes.float8_e4m3)
    q_mat[:, :, :QP] = q_hat.transpose(0, 2, 1, 3).reshape(B, D, QP)
    qp8 = np.ascontiguousarray(
        q_mat.reshape(B, ND, 128, QP_PAD).transpose(0, 2, 1, 3)).reshape(
        B, 128, ND * QP_PAD)

    # global-cosine branch on host (0.3% of the FLOPs)
    bmean = base.mean(axis=3)                                     # [B, way, D]
    bm = bmean / np.maximum(
        np.linalg.norm(bmean, axis=2, keepdims=True), GEPS)
    qmean = query.mean(axis=3)                                    # [B, nq, D]
    qm_hat = qmean / np.maximum(
        np.linalg.norm(qmean, axis=2, keepdims=True), GEPS)
    glob = np.einsum("bqd,bkd->bqk", qm_hat, bm)                  # [B, nq, way]

    # patch->query aggregation matrix (0/1), [128, NT*NQ]
    am = np.zeros((128, NT, NQ), dtype=np.float32)
    for t in range(NT):
        qp_idx = t * 128 + np.arange(128)
        valid = qp_idx < QP
        am[valid, t, qp_idx[valid] // HW] = 1.0
    am = am.reshape(128, NT * NQ).astype(ml_dtypes.bfloat16)

    if k not in _CACHE:
        _CACHE[k] = _build(k)
    nc = _CACHE[k]

    in_maps = []
    for c in range(N_CORES):
        sl = slice(c * EPC, (c + 1) * EPC)
        qc = qp8[sl]                                              # [EPC,128,ND*1920]
        in_maps.append({
            "qp8": qc,
            "q0h": np.ascontiguousarray(
                qc[0].reshape(128, ND, QP_PAD)[:, :, :HEAD]).reshape(
                128, ND * HEAD),
            "seh": seh[sl],
            "amat": am,
        })
    global _LAST_IN_MAPS
    _LAST_IN_MAPS = in_maps
    res = run_bass_kernel_spmd(nc, in_maps, list(range(N_CORES)))
    dev = np.stack([res.results[c]["out"] for c in range(N_CORES)])  # [C, EPC, WAY, NQ]
    dn4 = dev.reshape(B, WAY, NQ).transpose(0, 2, 1) / (QSCALE * k)  # [B, nq, way]
    return (r[0] * glob + r[1] * dn4).astype(np.float32)


# revision 14
# speedup vs baseline: 1.4215x; 1.0467x over previous
"""Trainium2 Bass kernel for nn_MetaBaseline (global-cosine + DN4 few-shot scoring).

Math (per episode b):
  global: logits[q,k] = <qmean_hat, bmean_hat>          (means over the 5x5 spatial grid)
  DN4:    sim[q,p,k,l] = <q_patch[q,p], s_col_hat[k,l]>  -> sum of top-neighbor_k over l,
          summed over p, / neighbor_k
  out = r0 * logits + r1 * dn4

Device strategy (data-parallel, 8 episodes per NeuronCore):
  - the global-cosine branch is 0.3% of the FLOPs and runs on the host
    (means + one small einsum); the device computes only the DN4 branch.
  - host pre-normalizes both sides: s_hat packed [128, nd*125] bf16 per
    episode and q_hat packed [128, nd*1920] (qp-major, zero-padded from 1875)
    per episode in fp8 e4m3 scaled by 16 (halves the dominant HBM stream and
    makes each episode one contiguous 9600B-per-partition DMA; the x16 rides
    through the whole DN4 branch and is divided out on the host).
  - PE: sim[qp, 0:125] = q_hat^T @ s_hat as 15 qp-tiles x 5 k-tiles of
    [128,128]x[128,125] fp8xbf16 matmuls; four qp-tiles share one fp32 PSUM
    bank [128,500] so one ACT copy covers four tiles; 6 sim banks let the PE
    run ~1.5 episodes ahead of DVE while its clock ramps up.
  - DVE Max8 gives the top-8 of each 25-value support-patch group in one op;
    a strided add-chain of the first neighbor_k per episode gives the
    per-(patch,class) DN4 terms (on GpSimd, which is otherwise idle; the
    last episode reduces per-group on DVE so the drain is short).
  - tiny matmuls against a 0/1 aggregation matrix contract the 25 patches of
    each query across partitions.
  - startup: episode 0 streams in three column chunks so the first matmul
    waits on ~0.5MB; later episodes prefetch 4 deep on two DMA queues.
  - host applies 1/(16*neighbor_k) and the r-weighted combine.
"""
import numpy as np
import ml_dtypes

N_CORES = 8
B, WAY, SHOT, D, H, W = 64, 5, 1, 640, 5, 5
NQ = 75
HW = H * W                 # 25
QP = NQ * HW               # 1875 query patches per episode
NT = 15                    # qp tiles of 128
QP_PAD = NT * 128          # 1920
ND = D // 128              # 5 contraction tiles
EPC = B // N_CORES         # 8 episodes per core
SCOLS = WAY * HW           # 125
GEPS = 1e-12               # eps of the global-cosine branch (torch F.normalize)
QSCALE = 16.0              # fp8 pre-scale for q_hat
HEAD = 512                 # episode-0 head columns (tiles 0-3)
C1 = 1280                  # episode-0 second chunk boundary

_CACHE = {}
_LAST_IN_MAPS = None


def _build(k: int):
    """Build + compile the SPMD NEFF for top-k = k (k <= 8)."""
    import concourse.bacc as bacc
    import concourse.mybir as mybir
    import concourse.tile as tile

    bf16 = mybir.dt.bfloat16
    fp8 = mybir.dt.float8e4
    f32 = mybir.dt.float32
    COPY = mybir.ActivationFunctionType.Copy

    nc = bacc.Bacc("TRN2", target_bir_lowering=False, debug=False)
    # per-episode packed layouts: one contiguous DMA per episode
    qp8 = nc.dram_tensor("qp8", [EPC, 128, ND * QP_PAD], fp8, kind="ExternalInput")
    q0h = nc.dram_tensor("q0h", [128, ND * HEAD], fp8, kind="ExternalInput")
    seh = nc.dram_tensor("seh", [EPC, 128, ND * SCOLS], bf16, kind="ExternalInput")
    amat = nc.dram_tensor("amat", [128, NT * NQ], bf16, kind="ExternalInput")
    out = nc.dram_tensor("out", [EPC, WAY, NQ], f32, kind="ExternalOutput")

    qv = qp8[:].rearrange("e p (d c) -> e p d c", d=ND)  # strided e0 chunk views

    with tile.TileContext(nc) as tc:
        with (
            tc.tile_pool(name="const", bufs=1) as cpool,
            tc.tile_pool(name="qe0", bufs=1) as e0pool,
            tc.tile_pool(name="q", bufs=3) as qpool,
            tc.tile_pool(name="simps", bufs=6, space="PSUM") as simpool,
            tc.tile_pool(name="acc", bufs=2, space="PSUM") as accpool,
            tc.tile_pool(name="simsb", bufs=8) as sbpool,
            tc.tile_pool(name="out8", bufs=3) as o8pool,
            tc.tile_pool(name="draw", bufs=3) as drpool,
            tc.tile_pool(name="osb", bufs=2) as opool,
        ):
            # heads: the first matmul waits on just these two DMAs
            seh_t = {0: cpool.tile([128, ND * SCOLS], bf16, tag="seh0",
                                   name="seh0")}
            nc.sync.dma_start(seh_t[0][:], seh[0])
            q0h_t = cpool.tile([128, ND * HEAD], fp8)
            nc.gpsimd.dma_start(q0h_t[:], q0h[:])
            # rest of episode 0 in two packed-chunk tiles (strided DRAM view)
            qe0b = e0pool.tile([128, ND * (C1 - HEAD)], fp8)
            nc.sync.dma_start(
                qe0b[:].rearrange("p (d c) -> p d c", d=ND),
                qv[0, :, :, HEAD:C1])
            qe0c = e0pool.tile([128, ND * (QP_PAD - C1)], fp8)
            nc.gpsimd.dma_start(
                qe0c[:].rearrange("p (d c) -> p d c", d=ND),
                qv[0, :, :, C1:QP_PAD])

            pending = []  # deferred tail: (e, draw)

            def emit_tail():
                if not pending:
                    return
                e, draw = pending.pop()
                dn4_ps = accpool.tile([WAY, NQ], f32, tag="dn4ps")
                for t in range(NT):
                    nc.tensor.matmul(
                        dn4_ps[:], draw[:, t * WAY:(t + 1) * WAY],
                        amat_t[:, t * NQ:(t + 1) * NQ],
                        start=(t == 0), stop=(t == NT - 1),
                    )
                osb = opool.tile([WAY, NQ], f32)
                nc.scalar.activation(osb[:], dn4_ps[:], COPY)
                nc.scalar.dma_start(out[e], osb[:])

            qts = {}
            for e in range(EPC):
                if e == 0:
                    def qsl(t, d):
                        c = t * 128
                        if c < HEAD:
                            return q0h_t[:, d * HEAD + c:d * HEAD + c + 128]
                        if c < C1:
                            w = C1 - HEAD
                            return qe0b[:, d * w + c - HEAD:d * w + c - HEAD + 128]
                        w = QP_PAD - C1
                        return qe0c[:, d * w + c - C1:d * w + c - C1 + 128]
                else:
                    # prefetch: se on gpsimd, q strictly FIFO on sync so
                    # earlier episodes never lose wire bandwidth to later ones
                    seh_t[e] = cpool.tile([128, ND * SCOLS], bf16,
                                          tag=f"seh{e}", name=f"seh{e}")
                    nc.gpsimd.dma_start(seh_t[e][:], seh[e])
                    qt = qpool.tile([128, ND * QP_PAD], fp8)
                    nc.sync.dma_start(qt[:], qp8[e])
                    qts[e] = qt

                    def qsl(t, d, qt=qt):
                        return qt[:, d * QP_PAD + t * 128:d * QP_PAD + t * 128 + 128]

                def ssl(d, e=e):
                    return seh_t[e][:, d * SCOLS:(d + 1) * SCOLS]

                last = e == EPC - 1
                out8 = o8pool.tile([128, NT * WAY * 8], bf16)
                o8 = out8[:].rearrange("p (g e) -> p g e", e=8)
                draw = drpool.tile([128, NT * WAY], bf16)
                if last:
                    dn4_ps = accpool.tile([WAY, NQ], f32, tag="dn4ps")
                # four tiles share one PSUM bank: 4*125 fp32 = 2000B
                groups = [(0, 4), (4, 8), (8, 12), (12, 15)]
                for gi, (t0, t1) in enumerate(groups):
                    w = (t1 - t0) * SCOLS
                    simps = simpool.tile([128, 500], f32, tag="simps")
                    for t in range(t0, t1):
                        off = (t - t0) * SCOLS
                        for d in range(ND):
                            nc.tensor.matmul(
                                simps[:, off:off + SCOLS],
                                qsl(t, d), ssl(d),
                                start=(d == 0), stop=(d == ND - 1),
                            )
                    simsb = sbpool.tile([128, 500], bf16)
                    nc.scalar.activation(simsb[:, 0:w], simps[:, 0:w], COPY)
                    if e == 0 and gi == 0:
                        # needed first at e1/gi1; issue now, off the hot window
                        amat_t = cpool.tile([128, NT * NQ], bf16)
                        nc.scalar.dma_start(amat_t[:], amat[:])
                    for t in range(t0, t1):
                        off = (t - t0) * SCOLS
                        for kk in range(WAY):
                            g = t * WAY + kk
                            nc.vector.max(
                                out8[:, g * 8:(g + 1) * 8],
                                simsb[:, off + kk * HW:off + (kk + 1) * HW],
                            )
                    if last:
                        # reduce + aggregate this group now: short drain
                        g0, g1 = t0 * WAY, t1 * WAY
                        with nc.allow_low_precision("bf16 top-k sums"):
                            nc.vector.reduce_sum(
                                draw[:, g0:g1], o8[:, g0:g1, 0:k],
                                axis=mybir.AxisListType.X)
                        for t in range(t0, t1):
                            nc.tensor.matmul(
                                dn4_ps[:], draw[:, t * WAY:(t + 1) * WAY],
                                amat_t[:, t * NQ:(t + 1) * NQ],
                                start=(t == 0), stop=(t == NT - 1),
                            )
                    if gi == 1:
                        emit_tail()  # previous episode's aggregation matmuls
                if last:
                    osb = opool.tile([WAY, NQ], f32)
                    nc.scalar.activation(osb[:], dn4_ps[:], COPY)
                    nc.scalar.dma_start(out[e], osb[:])
                else:
                    with nc.allow_low_precision("bf16 top-k sums"):
                        if k == 1:
                            nc.gpsimd.tensor_copy(draw[:], o8[:, :, 0])
                        else:
                            # GpSimd lacks free-axis reduce; strided add chain
                            nc.gpsimd.tensor_add(
                                draw[:], o8[:, :, 0], o8[:, :, 1])
                            for j in range(2, k):
                                nc.gpsimd.tensor_add(
                                    draw[:], draw[:], o8[:, :, j])
                    pending.append((e, draw))
            emit_tail()
    nc.compile()
    return nc


def kernel(base, query, r, neighbor_k):
    from concourse.bass_utils import run_bass_kernel_spmd

    k = int(neighbor_k)
    assert 1 <= k <= 8, f"top-k must fit the Max8 output, got {k}"
    base = np.asarray(base, dtype=np.float32).reshape(B, WAY, D, HW)
    query = np.asarray(query, dtype=np.float32).reshape(B, NQ, D, HW)
    r = np.asarray(r, dtype=np.float32)

    # ---- host prep (layout + normalization) ----
    # support: normalized columns packed per episode -> [B, 128, ND*125] bf16
    s_norm = base / np.linalg.norm(base, axis=2, keepdims=True)
    s_ext = s_norm.transpose(0, 2, 1, 3).reshape(B, ND, 128, SCOLS)
    seh = np.ascontiguousarray(s_ext.transpose(0, 2, 1, 3)).reshape(
        B, 128, ND * SCOLS).astype(ml_dtypes.bfloat16)

    # query: normalized patches scaled x16 in fp8, packed [B, 128, ND*1920]
    qn = np.sqrt(np.einsum("bqdp,bqdp->bqp", query, query))      # [B, nq, hw]
    q_hat = query * (QSCALE / qn[:, :, None, :])
    q_mat = np.zeros((B, D, QP_PAD), dtype=ml_dtypes.float8_e4m3)
    q_mat[:, :, :QP] = q_hat.transpose(0, 2, 1, 3).reshape(B, D, QP)
    qp8 = np.ascontiguousarray(
        q_mat.reshape(B, ND, 128, QP_PAD).transpose(0, 2, 1, 3)).reshape(
        B, 128, ND * QP_PAD)

    # global-cosine branch on host (0.3% of the FLOPs)
    bmean = base.mean(axis=3)                                     # [B, way, D]
    bm = bmean / np.maximum(
        np.linalg.norm(bmean, axis=2, keepdims=True), GEPS)
    qmean = query.mean(axis=3)                                    # [B, nq, D]
    qm_hat = qmean / np.maximum(
        np.linalg.norm(qmean, axis=2, keepdims=True), GEPS)
    glob = np.einsum("bqd,bkd->bqk", qm_hat, bm)                  # [B, nq, way]

    # patch->query aggregation matrix (0/1), [128, NT*NQ]
    am = np.zeros((128, NT, NQ), dtype=np.float32)
    for t in range(NT):
        qp_idx = t * 128 + np.arange(128)
        valid = qp_idx < QP
        am[valid, t, qp_idx[valid] // HW] = 1.0
    am = am.reshape(128, NT * NQ).astype(ml_dtypes.bfloat16)

    if k not in _CACHE:
        _CACHE[k] = _build(k)
    nc = _CACHE[k]

    in_maps = []
    for c in range(N_CORES):
        sl = slice(c * EPC, (c + 1) * EPC)
        qc = qp8[sl]                                              # [EPC,128,ND*1920]
        in_maps.append({
            "qp8": qc,
            "q0h": np.ascontiguousarray(
                qc[0].reshape(128, ND, QP_PAD)[:, :, :HEAD]).reshape(
                128, ND * HEAD),
            "seh": seh[sl],
            "amat": am,
        })
    global _LAST_IN_MAPS
    _LAST_IN_MAPS = in_maps
    res = run_bass_kernel_spmd(nc, in_maps, list(range(N_CORES)))
    dev = np.stack([res.results[c]["out"] for c in range(N_CORES)])  # [C, EPC, WAY, NQ]
    dn4 = dev.reshape(B, WAY, NQ).transpose(0, 2, 1) / (QSCALE * k)  # [B, nq, way]
    return (r[0] * glob + r[1] * dn4).astype(np.float32)
